# revision 21
# baseline (speedup 1.0000x reference)
"""Causal multi-head attention on 8 TRN2 NeuronCores.

Reference computation (fp32):
    q,k,v = x @ {Q,K,V}.T split into 16 heads of 64
    scores = q k^T / 8, causal mask, softmax
    out    = (attn @ v concat heads) @ W_o.T

Sharding: core c (0..7) takes batch b = c//4 and head group g = c%4
(heads 4g..4g+3, i.e. a 256-row slice of Q/K/V and a 256-column slice
of W_o). Each core produces a partial [T, D] output; the host sums the
4 partials per batch. No on-device collectives.

Per-core DRAM layout (host pre-transposes everything so every matmul
contraction dim lands on SBUF partitions; no on-device transposes):
    xT    [1024, 2048] = x[b].T
    wqT   [1024, 256]  = Q[slice].T         (lhsT for qT = wqT.T @ xT)
    wkT   [1024, 256]  = K[slice].T
    wvT   [1024, 260]  = V[slice].T with a zero column after each head
                         (the ones-column there, added via a rank-1
                         matmul, makes the PV matmul emit the softmax
                         denominator for free in row 64)
    woT   [256, 1024]  = W_o[:, slice].T    (rhs for out = oT.T @ woT)
    maskz [128, 2048]  4 diag-mask variants ([tk, tq]): variant k has
                       cols [0,128k) = 0, [128k,128k+128) = triangular
                       (f-128k >= p), rest = 1
    ones [1, 128], wv1 [1, 260] (1 at each head's ones-column),
    zeros [128, 1024] (E-tile init)

Attention is computed transposed (ST[tk, tq] = k-block . qT-chunk) so
softmax exp is elementwise (no max subtraction: scores ~ N(0,1), exp
cannot overflow) and PV needs no transposes. All matmuls run in
float32r (full PE rate at N>=256, ~1e-4 relative precision).
Normalization runs entirely off the PE critical path: DVE reciprocal
of the denominator row, GpSimd partition-broadcast, DVE multiply.
"""

import numpy as np

import concourse.bass as bass  # noqa: F401
import concourse.tile as tile
from concourse import bacc, mybir
from concourse.bass_utils import run_bass_kernel_spmd

F32 = mybir.dt.float32
F32R = mybir.dt.float32r
BF16 = mybir.dt.bfloat16
EXP = mybir.ActivationFunctionType.Exp

import os as _os
WDT = BF16 if _os.environ.get("MHA_DTYPE", "bf16") == "bf16" else F32R

N_CORES = 8
T = 2048          # sequence length
D = 1024          # model dim
HPC = 4           # heads per core
HD = 64           # head dim
DS = HPC * HD     # 256: per-core slice of D
VW = HPC * (HD + 1)  # 260: v tiles with ones-column per head
CH = 1024         # tq chunk width
NCH = T // CH     # chunks
NTB = T // 128    # 128-row t blocks
NDB = D // 128    # 128-row d blocks


def build_program():
    nc = bacc.Bacc("TRN2", target_bir_lowering=False, debug=False,
                   num_devices=N_CORES)
    xT_d = nc.dram_tensor("xT", [D, T], WDT, kind="ExternalInput").ap()
    wqT_d = nc.dram_tensor("wqT", [D, DS], WDT, kind="ExternalInput").ap()
    wkT_d = nc.dram_tensor("wkT", [D, DS], WDT, kind="ExternalInput").ap()
    wvT_d = nc.dram_tensor("wvT", [D, VW], WDT, kind="ExternalInput").ap()
    woT_d = nc.dram_tensor("woT", [DS, D], WDT, kind="ExternalInput").ap()
    maskz_d = nc.dram_tensor("maskz", [128, 2048], WDT,
                             kind="ExternalInput").ap()
    ones_d = nc.dram_tensor("ones", [1, 128], WDT, kind="ExternalInput").ap()
    wv1_d = nc.dram_tensor("wv1", [1, VW], WDT, kind="ExternalInput").ap()
    out_d = nc.dram_tensor("out", [T, D], F32, kind="ExternalOutput").ap()

    with tile.TileContext(nc) as tc, \
         tc.tile_pool(name="xt", bufs=16) as xt_pool, \
         tc.tile_pool(name="wq", bufs=8) as wq_pool, \
         tc.tile_pool(name="wk", bufs=8) as wk_pool, \
         tc.tile_pool(name="wv", bufs=8) as wv_pool, \
         tc.tile_pool(name="wo", bufs=2) as wo_pool, \
         tc.tile_pool(name="cst", bufs=1) as cst_pool, \
         tc.tile_pool(name="qk", bufs=16) as qk_pool, \
         tc.tile_pool(name="vv", bufs=16) as vv_pool, \
         tc.tile_pool(name="ot", bufs=4) as ot_pool, \
         tc.tile_pool(name="ee", bufs=4) as e_pool, \
         tc.tile_pool(name="rd", bufs=2) as rd_pool, \
         tc.tile_pool(name="ou", bufs=2) as ou_pool, \
         tc.tile_pool(name="rb", bufs=2) as rb_pool, \
         tc.tile_pool(name="ob", bufs=4) as ob_pool:

        # ---- tiny constants first (needed by stage-1 v matmuls) --------
        ones_t = cst_pool.tile([1, 128], WDT, tag="ones")
        nc.sync.dma_start(ones_t[:], ones_d[:])
        wv1_t = cst_pool.tile([1, VW], WDT, tag="wv1")
        nc.sync.dma_start(wv1_t[:], wv1_d[:])

        # ---- interleave x chunk 0 with weights so PE starts early ------
        xt_t = [[None] * NDB for _ in range(4)]
        wq_t, wk_t, wv_t = [], [], []
        for db in range(NDB):
            x = xt_pool.tile([128, 512], WDT, tag="xt", name=f"xt0_{db}")
            nc.sync.dma_start(x[:], xT_d[128 * db:128 * db + 128, 0:512])
            xt_t[0][db] = x
            w = wq_pool.tile([128, DS], WDT, tag="wq", name=f"wq{db}")
            nc.sync.dma_start(w[:], wqT_d[128 * db:128 * db + 128, :])
            wq_t.append(w)
            w = wk_pool.tile([128, DS], WDT, tag="wk", name=f"wk{db}")
            nc.sync.dma_start(w[:], wkT_d[128 * db:128 * db + 128, :])
            wk_t.append(w)
            w = wv_pool.tile([128, VW], WDT, tag="wv", name=f"wv{db}")
            nc.sync.dma_start(w[:], wvT_d[128 * db:128 * db + 128, :])
            wv_t.append(w)

        # ---- persistent E tiles (PV only reads exp-written regions) ----
        e_tiles = [e_pool.tile([128, CH], WDT, tag="ee", name=f"ee{i}")
                   for i in range(4)]

        # ---- stage 1: projections qT, kT (e on partitions), v (natural)
        qT_t = [[None] * 4 for _ in range(2)]
        kT_t = [[None] * 4 for _ in range(2)]
        v_t = [None] * NTB

        # oT_t[db][c]: [128, CH] attention outputs, d on partitions
        # (head h lives in tile h//2 rows 64*(h%2)..+64)
        oT_t = [[ot_pool.tile([128, CH], WDT, tag="ot", name=f"ot{d}_{c}")
                 for c in range(NCH)] for d in range(2)]
        state = {"eidx": 0}

        with tc.tile_pool(name="pst", bufs=2, space="PSUM") as pst_pool, \
             tc.tile_pool(name="pac", bufs=1, space="PSUM") as pac_pool:

            def emit_xt_dma(tch):
                for db in range(NDB):
                    x = xt_pool.tile([128, 512], WDT, tag="xt",
                                     name=f"xt{tch}_{db}")
                    nc.sync.dma_start(
                        x[:], xT_d[128 * db:128 * db + 128,
                                   512 * tch:512 * tch + 512])
                    xt_t[tch][db] = x

            def emit_qk_group(ps1_pool, tch, eb, wt, dst):
                xt = xt_t[tch]
                ps = ps1_pool.tile([128, 512], F32, tag="ps1",
                                   name=f"p1_{tch}_{eb}_{dst is kT_t}")
                for db in range(NDB):
                    nc.tensor.matmul(
                        ps[:], wt[db][:, 128 * eb:128 * eb + 128],
                        xt[db][:], start=(db == 0), stop=(db == NDB - 1))
                q = qk_pool.tile([128, 512], WDT, tag="qk",
                                 name=f"qk_{tch}_{eb}_{dst is kT_t}")
                nc.vector.tensor_copy(q[:], ps[:])
                dst[eb][tch] = q

            def emit_v_group(ps1_pool, tb):
                xt = xt_t[tb // 4]
                ps = ps1_pool.tile([128, VW], F32, tag="ps1",
                                   name=f"p1v_{tb}")
                for db in range(NDB):
                    nc.tensor.matmul(
                        ps[:],
                        xt[db][:, 128 * (tb % 4):128 * (tb % 4) + 128],
                        wv_t[db][:], start=(db == 0), stop=False)
                # ones-columns: rank-1 update 1s^T . wv1
                nc.tensor.matmul(ps[:], ones_t[:], wv1_t[:],
                                 start=False, stop=True)
                v = vv_pool.tile([128, VW], WDT, tag="vv", name=f"v{tb}")
                nc.vector.tensor_copy(v[:], ps[:])
                v_t[tb] = v

            def emit_stage1_tch(ps1_pool, tch):
                for wt, dst in ((wq_t, qT_t), (wk_t, kT_t)):
                    for eb in range(2):
                        emit_qk_group(ps1_pool, tch, eb, wt, dst)
                for tb in range(4 * tch, 4 * tch + 4):
                    emit_v_group(ps1_pool, tb)

            def emit_pv(acc, c, h, j, e, off):
                jmax = 8 * c + 7
                alg = (off // 512) * 512
                for s in range(alg, CH, 512):
                    lo = max(s, off)
                    nc.tensor.matmul(
                        acc[:, lo:s + 512],
                        v_t[j][:, 65 * h:65 * h + 65],
                        e[:, lo:s + 512],
                        start=(j == 0),
                        stop=(j == (8 * c + 3 if s == 0 else jmax)),
                    )

            def emit_pair(c, h):
                # attention for one (chunk, head), PV delayed 2 j-steps so
                # the exp (ACT) latency never stalls the PE stream
                pb, rw = h // 2, 64 * (h % 2)
                jmax = 8 * c + 7
                acc = pac_pool.tile([65, CH], F32, tag="pac",
                                    name=f"ac{c}_{h}")
                pending = []
                for j in range(jmax + 1):
                    off = max(0, 128 * j - CH * c)
                    alg = (off // 512) * 512  # 512-aligned ST psum base
                    st = pst_pool.tile([128, CH], F32, tag="pst",
                                       name=f"st{c}_{h}_{j}")
                    # ST[tk, tq] = k-block . qT-chunk
                    for s in range(alg, CH, 512):
                        lo = max(s, off)
                        nc.tensor.matmul(
                            st[:, lo:s + 512],
                            kT_t[pb][j // 4][
                                rw:rw + 64,
                                128 * (j % 4):128 * (j % 4) + 128],
                            qT_t[pb][2 * c + s // 512][rw:rw + 64,
                                                       lo - s:512],
                            start=True, stop=True)
                    e = e_tiles[state["eidx"] % len(e_tiles)]
                    state["eidx"] += 1
                    nc.scalar.activation(e[:, off:], st[:, off:], EXP,
                                         scale=0.125)
                    if 128 * j >= CH * c:
                        # diagonal block: tri mask (PV reads from off on,
                        # so below-diagonal cols never need zeroing)
                        nc.vector.tensor_mul(
                            e[:, off:off + 128], e[:, off:off + 128],
                            maskz_t[:, 0:128])
                    pending.append((j, e, off))
                    if len(pending) > 2:
                        jd, ed, ad = pending.pop(0)
                        emit_pv(acc, c, h, jd, ed, ad)
                for jd, ed, ad in pending:
                    emit_pv(acc, c, h, jd, ed, ad)
                # normalization, entirely off the PE critical path: one
                # whole-acc copy frees PSUM, then approx-reciprocal of the
                # denominator row, GpSimd partition-broadcast, multiply.
                oTu = ou_pool.tile([65, CH], F32, tag="ou")
                nc.vector.tensor_copy(oTu[:], acc[:])
                den0 = rd_pool.tile([1, CH], F32, tag="dn")
                nc.sync.dma_start(den0[:], oTu[64:65, :])
                rden = rd_pool.tile([1, CH], F32, tag="rd")
                nc.vector.reciprocal_approx_fast(rden[:], den0[:])
                rbt = rb_pool.tile([128, CH], F32, tag="rb")
                for s in range(0, CH, 512):
                    nc.gpsimd.partition_broadcast(rbt[:, s:s + 512],
                                                  rden[:, s:s + 512])
                    nc.vector.tensor_mul(
                        oT_t[pb][c][rw:rw + 64, s:s + 512],
                        oTu[0:64, s:s + 512], rbt[0:64, s:s + 512])

            with tc.tile_pool(name="ps1", bufs=2, space="PSUM") as ps1_pool:
                emit_xt_dma(1)
                maskz_t = cst_pool.tile([128, 2048], WDT, tag="maskz")
                nc.sync.dma_start(maskz_t[:], maskz_d[:])
                wo_t = []
                for db in range(DS // 128):
                    w = wo_pool.tile([128, D], WDT, tag="wo",
                                     name=f"wo{db}")
                    nc.sync.dma_start(w[:], woT_d[128 * db:128 * db + 128, :])
                    wo_t.append(w)
                emit_xt_dma(2)
                emit_xt_dma(3)
                emit_stage1_tch(ps1_pool, 0)
                emit_stage1_tch(ps1_pool, 1)
                # c=0 attention interleaved with the rest of stage 1:
                # stage-1 matmul groups keep the PE dense while ACT
                # works through the exp stream
                fillers = ([("qk", 2, eb, wt, dst)
                            for wt, dst in ((wq_t, qT_t), (wk_t, kT_t))
                            for eb in range(2)]
                           + [("v", tb) for tb in range(8, 12)]
                           + [("qk", 3, eb, wt, dst)
                              for wt, dst in ((wq_t, qT_t), (wk_t, kT_t))
                              for eb in range(2)]
                           + [("v", tb) for tb in range(12, 16)])
                for h in range(HPC):
                    emit_pair(0, h)
                    for f in fillers[4 * h:4 * h + 4]:
                        if f[0] == "qk":
                            emit_qk_group(ps1_pool, f[1], f[2], f[3], f[4])
                        else:
                            emit_v_group(ps1_pool, f[1])

            def emit_stage5(ps5_pool, tb):
                c, tw = tb // 8, 128 * (tb % 8)
                pss = [ps5_pool.tile([128, 512], F32, tag="ps5",
                                     name=f"ps5_{tb}_{eb}")
                       for eb in range(2)]
                for db in range(2):
                    for eb in range(2):
                        nc.tensor.matmul(
                            pss[eb][:], oT_t[db][c][:, tw:tw + 128],
                            wo_t[db][:, 512 * eb:512 * eb + 512],
                            start=(db == 0), stop=(db == 1))
                for eb in range(2):
                    ob = ob_pool.tile([128, 512], F32, tag="ob")
                    nc.vector.tensor_copy(ob[:], pss[eb][:])
                    nc.sync.dma_start(
                        out_d[128 * tb:128 * tb + 128,
                              512 * eb:512 * eb + 512], ob[:])

            with tc.tile_pool(name="ps5", bufs=2, space="PSUM") as ps5_pool:
                # c=1 attention interleaved with stage-5 on the finished
                # c=0 chunk
                for h in range(HPC):
                    emit_pair(1, h)
                    emit_stage5(ps5_pool, 2 * h)
                    emit_stage5(ps5_pool, 2 * h + 1)
                for tb in range(8, NTB):
                    emit_stage5(ps5_pool, tb)

    nc.compile()
    return nc


_PROG = None


def _get_prog():
    global _PROG
    if _PROG is None:
        _PROG = build_program()
    return _PROG


def make_in_maps(x, Q, K, V, W_o):
    np_dt = mybir.dt.np(WDT)
    B = x.shape[0]
    maskz = np.zeros((128, 2048), dtype=np.float32)
    for k in range(4):
        blk = maskz[:, 512 * k:512 * k + 512]
        blk[:, 128 * k + 128:] = 1.0
        blk[:, 128 * k:128 * k + 128] = np.greater_equal(
            np.arange(128)[None, :], np.arange(128)[:, None])
    maskz = maskz.astype(np_dt)
    ones = np.ones((1, 128), dtype=np_dt)
    wv1 = np.zeros((1, VW), dtype=np.float32)
    wv1[0, 64::65] = 1.0
    wv1 = wv1.astype(np_dt)

    in_maps = []
    for c in range(N_CORES):
        b, g = divmod(c, N_CORES // B)
        sl = slice(DS * g, DS * g + DS)
        wvT = V[sl, :].T  # [D, 256]
        wvT_pad = np.zeros((D, VW), dtype=np.float32)
        for h in range(HPC):
            wvT_pad[:, 65 * h:65 * h + 64] = wvT[:, 64 * h:64 * h + 64]
        in_maps.append({
            "xT": np.ascontiguousarray(x[b].T).astype(np_dt),
            "wqT": np.ascontiguousarray(Q[sl, :].T).astype(np_dt),
            "wkT": np.ascontiguousarray(K[sl, :].T).astype(np_dt),
            "wvT": wvT_pad.astype(np_dt),
            "woT": np.ascontiguousarray(W_o[:, sl].T).astype(np_dt),
            "maskz": maskz,
            "ones": ones,
            "wv1": wv1,
        })
    return in_maps


def kernel(x, Q, K, V, W_o):
    x = np.asarray(x, dtype=np.float32)
    Q = np.asarray(Q, dtype=np.float32)
    K = np.asarray(K, dtype=np.float32)
    V = np.asarray(V, dtype=np.float32)
    W_o = np.asarray(W_o, dtype=np.float32)

    nc = _get_prog()
    in_maps = make_in_maps(x, Q, K, V, W_o)
    res = run_bass_kernel_spmd(nc, in_maps, core_ids=list(range(N_CORES)))

    B = x.shape[0]
    out = np.zeros((B, T, D), dtype=np.float32)
    for c in range(N_CORES):
        out[c // (N_CORES // B)] += res.results[c]["out"]
    return out


# revision 22
# speedup vs baseline: 1.0051x; 1.0051x over previous
"""Causal multi-head attention on 8 TRN2 NeuronCores.

Reference computation (fp32):
    q,k,v = x @ {Q,K,V}.T split into 16 heads of 64
    scores = q k^T / 8, causal mask, softmax
    out    = (attn @ v concat heads) @ W_o.T

Sharding: core c (0..7) takes batch b = c//4 and head group g = c%4
(heads 4g..4g+3, i.e. a 256-row slice of Q/K/V and a 256-column slice
of W_o). Each core produces a partial [T, D] output; the host sums the
4 partials per batch. No on-device collectives.

Per-core DRAM layout (host pre-transposes everything so every matmul
contraction dim lands on SBUF partitions; no on-device transposes):
    xT    [1024, 2048] = x[b].T
    wqT   [1024, 256]  = Q[slice].T         (lhsT for qT = wqT.T @ xT)
    wkT   [1024, 256]  = K[slice].T
    wvT   [1024, 260]  = V[slice].T with a zero column after each head
                         (the ones-column there, added via a rank-1
                         matmul, makes the PV matmul emit the softmax
                         denominator for free in row 64)
    woT   [256, 1024]  = W_o[:, slice].T    (rhs for out = oT.T @ woT)
    maskz [128, 2048]  mask constants; only [:, 0:128] (triangular
                       f >= p, [tk, tq] orientation) is used
    ones [1, 128], wv1 [1, 260] (1 at each head's ones-column)

Attention is computed transposed (ST[tk, tq] = k-block . qT-chunk) so
softmax exp is elementwise (no max subtraction: scores ~ N(0,1), exp
cannot overflow) and PV needs no transposes; exp runs on ACT straight
out of PSUM. The schedule keeps the PE stream dense so the HAM clock
gate stays at 2.4 GHz: PV lags ST by two j-steps (hiding exp latency),
stage-1's second half fills the c=0 attention chunk, and stage-5 fills
the c=1 chunk. Softmax normalization runs entirely off the PE critical
path (one acc copy frees PSUM, then approx-reciprocal, GpSimd
partition-broadcast, and a multiply).
"""

import numpy as np

import concourse.bass as bass  # noqa: F401
import concourse.tile as tile
from concourse import bacc, mybir
from concourse.bass_utils import run_bass_kernel_spmd

F32 = mybir.dt.float32
F32R = mybir.dt.float32r
BF16 = mybir.dt.bfloat16
EXP = mybir.ActivationFunctionType.Exp

import os as _os

# matmul operand dtype: bf16 (full PE rate + fast weight load; all
# accumulations stay in fp32 PSUM and softmax denominators are computed
# in fp32, so the only loss is bf16 input/intermediate rounding,
# ~4e-3 relative). Set MHA_DTYPE=f32r for ~2e-4 at ~20% more time.
WDT = BF16 if _os.environ.get("MHA_DTYPE", "bf16") == "bf16" else F32R

N_CORES = 8
T = 2048          # sequence length
D = 1024          # model dim
HPC = 4           # heads per core
HD = 64           # head dim
DS = HPC * HD     # 256: per-core slice of D
VW = HPC * (HD + 1)  # 260: v tiles with ones-column per head
CH = 1024         # tq chunk width
NCH = T // CH     # chunks
NTB = T // 128    # 128-row t blocks
NDB = D // 128    # 128-row d blocks


def build_program():
    nc = bacc.Bacc("TRN2", target_bir_lowering=False, debug=False,
                   num_devices=N_CORES)
    xT_d = nc.dram_tensor("xT", [D, T], WDT, kind="ExternalInput").ap()
    wqT_d = nc.dram_tensor("wqT", [D, DS], WDT, kind="ExternalInput").ap()
    wkT_d = nc.dram_tensor("wkT", [D, DS], WDT, kind="ExternalInput").ap()
    wvT_d = nc.dram_tensor("wvT", [D, VW], WDT, kind="ExternalInput").ap()
    woT_d = nc.dram_tensor("woT", [DS, D], WDT, kind="ExternalInput").ap()
    maskz_d = nc.dram_tensor("maskz", [128, 2048], WDT,
                             kind="ExternalInput").ap()
    ones_d = nc.dram_tensor("ones", [1, 128], WDT, kind="ExternalInput").ap()
    wv1_d = nc.dram_tensor("wv1", [1, VW], WDT, kind="ExternalInput").ap()
    out_d = nc.dram_tensor("out", [T, D], F32, kind="ExternalOutput").ap()

    with tile.TileContext(nc) as tc, \
         tc.tile_pool(name="xt", bufs=16) as xt_pool, \
         tc.tile_pool(name="wq", bufs=8) as wq_pool, \
         tc.tile_pool(name="wk", bufs=8) as wk_pool, \
         tc.tile_pool(name="wv", bufs=8) as wv_pool, \
         tc.tile_pool(name="wo", bufs=2) as wo_pool, \
         tc.tile_pool(name="cst", bufs=1) as cst_pool, \
         tc.tile_pool(name="qk", bufs=16) as qk_pool, \
         tc.tile_pool(name="vv", bufs=16) as vv_pool, \
         tc.tile_pool(name="ot", bufs=4) as ot_pool, \
         tc.tile_pool(name="ee", bufs=4) as e_pool, \
         tc.tile_pool(name="rd", bufs=2) as rd_pool, \
         tc.tile_pool(name="ou", bufs=2) as ou_pool, \
         tc.tile_pool(name="rb", bufs=2) as rb_pool, \
         tc.tile_pool(name="ob", bufs=4) as ob_pool:

        # ---- tiny constants first (needed by stage-1 v matmuls) --------
        ones_t = cst_pool.tile([1, 128], WDT, tag="ones")
        nc.sync.dma_start(ones_t[:], ones_d[:])
        wv1_t = cst_pool.tile([1, VW], WDT, tag="wv1")
        nc.sync.dma_start(wv1_t[:], wv1_d[:])

        # ---- interleave x chunk 0 with weights so PE starts early ------
        xt_t = [[None] * NDB for _ in range(4)]
        wq_t, wk_t, wv_t = [], [], []
        for db in range(NDB):
            x = xt_pool.tile([128, 512], WDT, tag="xt", name=f"xt0_{db}")
            nc.sync.dma_start(x[:], xT_d[128 * db:128 * db + 128, 0:512])
            xt_t[0][db] = x
            w = wq_pool.tile([128, DS], WDT, tag="wq", name=f"wq{db}")
            nc.sync.dma_start(w[:], wqT_d[128 * db:128 * db + 128, :])
            wq_t.append(w)
            w = wk_pool.tile([128, DS], WDT, tag="wk", name=f"wk{db}")
            nc.sync.dma_start(w[:], wkT_d[128 * db:128 * db + 128, :])
            wk_t.append(w)
            w = wv_pool.tile([128, VW], WDT, tag="wv", name=f"wv{db}")
            nc.sync.dma_start(w[:], wvT_d[128 * db:128 * db + 128, :])
            wv_t.append(w)

        # ---- persistent E tiles (PV only reads exp-written regions) ----
        e_tiles = [e_pool.tile([128, CH], WDT, tag="ee", name=f"ee{i}")
                   for i in range(4)]

        # ---- stage 1: projections qT, kT (e on partitions), v (natural)
        qT_t = [[None] * 4 for _ in range(2)]
        kT_t = [[None] * 4 for _ in range(2)]
        v_t = [None] * NTB

        # oT_t[db][c]: [128, CH] attention outputs, d on partitions
        # (head h lives in tile h//2 rows 64*(h%2)..+64)
        oT_t = [[ot_pool.tile([128, CH], WDT, tag="ot", name=f"ot{d}_{c}")
                 for c in range(NCH)] for d in range(2)]
        state = {"eidx": 0}

        with tc.tile_pool(name="pst", bufs=2, space="PSUM") as pst_pool, \
             tc.tile_pool(name="pac", bufs=1, space="PSUM") as pac_pool:

            def emit_xt_dma(tch):
                for db in range(NDB):
                    x = xt_pool.tile([128, 512], WDT, tag="xt",
                                     name=f"xt{tch}_{db}")
                    nc.sync.dma_start(
                        x[:], xT_d[128 * db:128 * db + 128,
                                   512 * tch:512 * tch + 512])
                    xt_t[tch][db] = x

            def emit_qk_group(ps1_pool, tch, eb, wt, dst):
                xt = xt_t[tch]
                ps = ps1_pool.tile([128, 512], F32, tag="ps1",
                                   name=f"p1_{tch}_{eb}_{dst is kT_t}")
                for db in range(NDB):
                    nc.tensor.matmul(
                        ps[:], wt[db][:, 128 * eb:128 * eb + 128],
                        xt[db][:], start=(db == 0), stop=(db == NDB - 1))
                q = qk_pool.tile([128, 512], WDT, tag="qk",
                                 name=f"qk_{tch}_{eb}_{dst is kT_t}")
                nc.vector.tensor_copy(q[:], ps[:])
                dst[eb][tch] = q

            def emit_v_group(ps1_pool, tb):
                xt = xt_t[tb // 4]
                ps = ps1_pool.tile([128, VW], F32, tag="ps1",
                                   name=f"p1v_{tb}")
                for db in range(NDB):
                    nc.tensor.matmul(
                        ps[:],
                        xt[db][:, 128 * (tb % 4):128 * (tb % 4) + 128],
                        wv_t[db][:], start=(db == 0), stop=False)
                # ones-columns: rank-1 update 1s^T . wv1
                nc.tensor.matmul(ps[:], ones_t[:], wv1_t[:],
                                 start=False, stop=True)
                v = vv_pool.tile([128, VW], WDT, tag="vv", name=f"v{tb}")
                nc.vector.tensor_copy(v[:], ps[:])
                v_t[tb] = v

            def emit_stage1_tch(ps1_pool, tch):
                for wt, dst in ((wq_t, qT_t), (wk_t, kT_t)):
                    for eb in range(2):
                        emit_qk_group(ps1_pool, tch, eb, wt, dst)
                for tb in range(4 * tch, 4 * tch + 4):
                    emit_v_group(ps1_pool, tb)

            def emit_pv(acc, c, h, j, e, off):
                jmax = 8 * c + 7
                alg = (off // 512) * 512
                for s in range(alg, CH, 512):
                    lo = max(s, off)
                    nc.tensor.matmul(
                        acc[:, lo:s + 512],
                        v_t[j][:, 65 * h:65 * h + 65],
                        e[:, lo:s + 512],
                        start=(j == 0),
                        stop=(j == (8 * c + 3 if s == 0 else jmax)),
                    )

            def emit_pair(c, h):
                # attention for one (chunk, head), PV delayed 2 j-steps so
                # the exp (ACT) latency never stalls the PE stream
                pb, rw = h // 2, 64 * (h % 2)
                jmax = 8 * c + 7
                acc = pac_pool.tile([65, CH], F32, tag="pac",
                                    name=f"ac{c}_{h}")
                pending = []
                for j in range(jmax + 1):
                    off = max(0, 128 * j - CH * c)
                    alg = (off // 512) * 512  # 512-aligned ST psum base
                    st = pst_pool.tile([128, CH], F32, tag="pst",
                                       name=f"st{c}_{h}_{j}")
                    # ST[tk, tq] = k-block . qT-chunk
                    for s in range(alg, CH, 512):
                        lo = max(s, off)
                        nc.tensor.matmul(
                            st[:, lo:s + 512],
                            kT_t[pb][j // 4][
                                rw:rw + 64,
                                128 * (j % 4):128 * (j % 4) + 128],
                            qT_t[pb][2 * c + s // 512][rw:rw + 64,
                                                       lo - s:512],
                            start=True, stop=True)
                    e = e_tiles[state["eidx"] % len(e_tiles)]
                    state["eidx"] += 1
                    nc.scalar.activation(e[:, off:], st[:, off:], EXP,
                                         scale=0.125)
                    if 128 * j >= CH * c:
                        # diagonal block: tri mask (PV reads from off on,
                        # so below-diagonal cols never need zeroing)
                        nc.vector.tensor_mul(
                            e[:, off:off + 128], e[:, off:off + 128],
                            maskz_t[:, 0:128])
                    pending.append((j, e, off))
                    if len(pending) > 2:
                        jd, ed, ad = pending.pop(0)
                        emit_pv(acc, c, h, jd, ed, ad)
                for jd, ed, ad in pending:
                    emit_pv(acc, c, h, jd, ed, ad)
                # normalization, entirely off the PE critical path: one
                # whole-acc copy frees PSUM, then approx-reciprocal of the
                # denominator row, GpSimd partition-broadcast, multiply.
                oTu = ou_pool.tile([65, CH], F32, tag="ou")
                nc.vector.tensor_copy(oTu[:], acc[:])
                den0 = rd_pool.tile([1, CH], F32, tag="dn")
                nc.sync.dma_start(den0[:], oTu[64:65, :])
                rden = rd_pool.tile([1, CH], F32, tag="rd")
                nc.vector.reciprocal_approx_fast(rden[:], den0[:])
                rbt = rb_pool.tile([128, CH], F32, tag="rb")
                for s in range(0, CH, 512):
                    nc.gpsimd.partition_broadcast(rbt[:, s:s + 512],
                                                  rden[:, s:s + 512])
                    nc.vector.tensor_mul(
                        oT_t[pb][c][rw:rw + 64, s:s + 512],
                        oTu[0:64, s:s + 512], rbt[0:64, s:s + 512])

            with tc.tile_pool(name="ps1", bufs=2, space="PSUM") as ps1_pool:
                emit_xt_dma(1)
                maskz_t = cst_pool.tile([128, 2048], WDT, tag="maskz")
                nc.sync.dma_start(maskz_t[:], maskz_d[:])
                wo_t = []
                for db in range(DS // 128):
                    w = wo_pool.tile([128, D], WDT, tag="wo",
                                     name=f"wo{db}")
                    nc.sync.dma_start(w[:], woT_d[128 * db:128 * db + 128, :])
                    wo_t.append(w)
                emit_xt_dma(2)
                emit_xt_dma(3)
                emit_stage1_tch(ps1_pool, 0)
                emit_stage1_tch(ps1_pool, 1)
                # c=0 attention interleaved with the rest of stage 1:
                # stage-1 matmul groups keep the PE dense while ACT
                # works through the exp stream
                fillers = ([("qk", 2, eb, wt, dst)
                            for wt, dst in ((wq_t, qT_t), (wk_t, kT_t))
                            for eb in range(2)]
                           + [("v", tb) for tb in range(8, 12)]
                           + [("qk", 3, eb, wt, dst)
                              for wt, dst in ((wq_t, qT_t), (wk_t, kT_t))
                              for eb in range(2)]
                           + [("v", tb) for tb in range(12, 16)])
                for h in range(HPC):
                    emit_pair(0, h)
                    for f in fillers[4 * h:4 * h + 4]:
                        if f[0] == "qk":
                            emit_qk_group(ps1_pool, f[1], f[2], f[3], f[4])
                        else:
                            emit_v_group(ps1_pool, f[1])

            def emit_stage5(ps5_pool, tb):
                c, tw = tb // 8, 128 * (tb % 8)
                pss = [ps5_pool.tile([128, 512], F32, tag="ps5",
                                     name=f"ps5_{tb}_{eb}")
                       for eb in range(2)]
                for db in range(2):
                    for eb in range(2):
                        nc.tensor.matmul(
                            pss[eb][:], oT_t[db][c][:, tw:tw + 128],
                            wo_t[db][:, 512 * eb:512 * eb + 512],
                            start=(db == 0), stop=(db == 1))
                for eb in range(2):
                    ob = ob_pool.tile([128, 512], F32, tag="ob")
                    nc.vector.tensor_copy(ob[:], pss[eb][:])
                    nc.sync.dma_start(
                        out_d[128 * tb:128 * tb + 128,
                              512 * eb:512 * eb + 512], ob[:])

            with tc.tile_pool(name="ps5", bufs=2, space="PSUM") as ps5_pool:
                # c=1 attention interleaved with stage-5 on the finished
                # c=0 chunk
                for h in range(HPC):
                    emit_pair(1, h)
                    emit_stage5(ps5_pool, 2 * h)
                    emit_stage5(ps5_pool, 2 * h + 1)
                for tb in range(8, NTB):
                    emit_stage5(ps5_pool, tb)

    nc.compile()
    return nc


_PROG = None


def _get_prog():
    global _PROG
    if _PROG is None:
        _PROG = build_program()
    return _PROG


def make_in_maps(x, Q, K, V, W_o):
    np_dt = mybir.dt.np(WDT)
    B = x.shape[0]
    maskz = np.zeros((128, 2048), dtype=np.float32)
    for k in range(4):
        blk = maskz[:, 512 * k:512 * k + 512]
        blk[:, 128 * k + 128:] = 1.0
        blk[:, 128 * k:128 * k + 128] = np.greater_equal(
            np.arange(128)[None, :], np.arange(128)[:, None])
    maskz = maskz.astype(np_dt)
    ones = np.ones((1, 128), dtype=np_dt)
    wv1 = np.zeros((1, VW), dtype=np.float32)
    wv1[0, 64::65] = 1.0
    wv1 = wv1.astype(np_dt)

    in_maps = []
    for c in range(N_CORES):
        b, g = divmod(c, N_CORES // B)
        sl = slice(DS * g, DS * g + DS)
        wvT = V[sl, :].T  # [D, 256]
        wvT_pad = np.zeros((D, VW), dtype=np.float32)
        for h in range(HPC):
            wvT_pad[:, 65 * h:65 * h + 64] = wvT[:, 64 * h:64 * h + 64]
        in_maps.append({
            "xT": np.ascontiguousarray(x[b].T).astype(np_dt),
            "wqT": np.ascontiguousarray(Q[sl, :].T).astype(np_dt),
            "wkT": np.ascontiguousarray(K[sl, :].T).astype(np_dt),
            "wvT": wvT_pad.astype(np_dt),
            "woT": np.ascontiguousarray(W_o[:, sl].T).astype(np_dt),
            "maskz": maskz,
            "ones": ones,
            "wv1": wv1,
        })
    return in_maps


def kernel(x, Q, K, V, W_o):
    x = np.asarray(x, dtype=np.float32)
    Q = np.asarray(Q, dtype=np.float32)
    K = np.asarray(K, dtype=np.float32)
    V = np.asarray(V, dtype=np.float32)
    W_o = np.asarray(W_o, dtype=np.float32)

    nc = _get_prog()
    in_maps = make_in_maps(x, Q, K, V, W_o)
    res = run_bass_kernel_spmd(nc, in_maps, core_ids=list(range(N_CORES)))

    B = x.shape[0]
    out = np.zeros((B, T, D), dtype=np.float32)
    for c in range(N_CORES):
        out[c // (N_CORES // B)] += res.results[c]["out"]
    return out
